# revision 1
# baseline (speedup 1.0000x reference)
"""Trainium2 Bass kernel for the self-attention module:

    f = conv1x1(x)            # [B, 16, N]   (w1 @ x + b1)
    E = f^T f                 # [B, N, N]    (symmetric)
    A = softmax(E, axis=-1)
    y = x + 0.1 * (x @ A^T)   # out[b,c,n] = sum_m x[b,c,m] A[b,n,m]

Sharding: 8 cores = 4 batches x 2 halves of the N=4096 rows. Each core
gets the full x[b] (column-rolled so its 2048-row half sits first) and
produces y[:, :2048] for that layout.

Device algorithm per core (all matmuls fp32r = single-pass FP22):
  - f = w1T^T @ x + b1                          [16, 4096]
  - per 512-wide n-block, per 128-wide m-chunk:
      E_psum[m,n]  = f[:,m]^T @ f[:,n]          (K=16)
      P_sbuf       = exp(E_psum)                (ScalarE; no max-sub:
                                                 E<=~54 so exp<3e23 fits
                                                 fp32, and the row max
                                                 cancels in the ratio)
      out_psum[c,n]  += xT[m,c]^T @ P           (c in 2 chunks of 128)
      cs_psum[1,n]   += tens^T @ P              (tens = 10.0-vector ->
                                                 cs = 10*colsum)
  - y = x + out * reciprocal(broadcast(cs))     (= x + 0.1*out/colsum)
"""

import numpy as np

B, C, N = 4, 256, 64 * 64
K = 16
HALF = N // 2          # rows per core
NB = HALF // 512       # 4 n-blocks of 512
MC = N // 128          # 32 m-chunks of 128
N_CORES = 8

_CACHE: dict = {}


def _emit_body(nc, sb1, sbp, sbo, sbe, ps_f, ps_e, ps_o, ps_c,
               x_d, xT_d, w1T_d, b1_d, y_d, f32, f32r, AF):
    # ---- load inputs: small first, xT on the ACT HWDGE ring ----
    w1T = sb1.tile([128, 2, K], f32r, tag="w1T")
    nc.sync.dma_start(out=w1T,
                      in_=w1T_d.rearrange("(cc p) k -> p cc k", p=128).bitcast(f32r))
    b1 = sb1.tile([K, 1], f32, tag="b1")
    nc.sync.dma_start(out=b1, in_=b1_d)
    xf0 = sb1.tile([128, N], f32r, tag="xf0")
    xf1 = sb1.tile([128, N], f32r, tag="xf1")
    for h in range(2):
        hsl = slice(h * (N // 2), (h + 1) * (N // 2))
        nc.sync.dma_start(out=xf0[:, hsl], in_=x_d[0:128, hsl].bitcast(f32r))
        nc.sync.dma_start(out=xf1[:, hsl], in_=x_d[128:256, hsl].bitcast(f32r))
    xT = sb1.tile([128, MC, C], f32r, tag="xT")
    nc.scalar.dma_start(out=xT,
                        in_=xT_d.rearrange("(j p) c -> p j c", p=128).bitcast(f32r))
    tens_f = sb1.tile([128, 1], f32, tag="tens_f")
    nc.vector.memset(tens_f, 10.0)
    tens = sb1.tile([128, 1], f32r, tag="tens")
    nc.vector.tensor_copy(out=tens, in_=tens_f)

    # ---- f = w1 @ x + b1 : [K, N] ----
    f_sb = sb1.tile([K, N], f32r, tag="f")
    for mj in range(N // 512):
        fp = ps_c.tile([K, 512], f32, tag="cs")
        nc.tensor.matmul(fp, lhsT=w1T[:, 0, :],
                         rhs=xf0[:, mj * 512:(mj + 1) * 512],
                         start=True, stop=False)
        nc.tensor.matmul(fp, lhsT=w1T[:, 1, :],
                         rhs=xf1[:, mj * 512:(mj + 1) * 512],
                         start=False, stop=True)
        nc.vector.tensor_scalar_add(
            out=f_sb[:, mj * 512:(mj + 1) * 512], in0=fp, scalar1=b1)

    # ---- main: attention ----
    for j in range(NB):
        nsl = slice(j * 512, (j + 1) * 512)
        out0 = ps_o.tile([128, 512], f32, tag="c0")
        out1 = ps_o.tile([128, 512], f32, tag="c1")
        cs = ps_c.tile([1, 512], f32, tag="cs")
        for i in range(MC):
            ep = ps_e.tile([128, 512], f32, tag="e")
            nc.tensor.matmul(ep,
                             lhsT=f_sb[:, i * 128:(i + 1) * 128],
                             rhs=f_sb[:, nsl],
                             start=True, stop=True)
            p = sbp.tile([128, 512], f32r, tag="p")
            nc.scalar.activation(out=p, in_=ep, func=AF.Exp)
            nc.tensor.matmul(out0, lhsT=xT[:, i, 0:128],
                             rhs=p, start=(i == 0), stop=(i == MC - 1))
            nc.tensor.matmul(out1, lhsT=xT[:, i, 128:256],
                             rhs=p, start=(i == 0), stop=(i == MC - 1))
            nc.tensor.matmul(cs, lhsT=tens, rhs=p,
                             start=(i == 0), stop=(i == MC - 1))
        # epilogue: y[:, nsl] = x[:, nsl] + out * (0.1 / colsum)
        # broadcast 10*colsum across partitions with a stride-0 DMA read
        cs_sb = sbe.tile([128, 512], f32, tag="cs_sb")
        nc.vector.tensor_copy(out=cs_sb[0:1, :], in_=cs)
        bcast = sbe.tile([128, 512], f32, tag="bcast")
        nc.gpsimd.partition_broadcast(out_ap=bcast[:], in_ap=cs_sb[:],
                                      channels=128)
        rec = sbe.tile([128, 512], f32, tag="rec")
        nc.vector.reciprocal(out=rec, in_=bcast)
        for cc, outp, xfc in ((0, out0, xf0), (1, out1, xf1)):
            yo = sbo.tile([128, 512], f32, tag="yo")
            nc.vector.tensor_mul(yo, outp, rec)
            nc.vector.tensor_add(yo, yo, xfc[:, nsl].bitcast(f32))
            nc.sync.dma_start(out=y_d[cc * 128:(cc + 1) * 128, nsl], in_=yo)


def _build(loop_reps=None):
    from contextlib import ExitStack

    import concourse.mybir as mybir
    import concourse.tile as tile
    from concourse import bacc

    f32 = mybir.dt.float32
    f32r = mybir.dt.float32r
    AF = mybir.ActivationFunctionType

    nc = bacc.Bacc("TRN2", target_bir_lowering=False, debug=False,
                   num_devices=N_CORES)
    x_d = nc.dram_tensor("x", [C, N], f32, kind="ExternalInput").ap()
    xT_d = nc.dram_tensor("xT", [N, C], f32, kind="ExternalInput").ap()
    w1T_d = nc.dram_tensor("w1T", [C, K], f32, kind="ExternalInput").ap()
    b1_d = nc.dram_tensor("b1", [K, 1], f32, kind="ExternalInput").ap()
    y_d = nc.dram_tensor("y", [C, HALF], f32, kind="ExternalOutput").ap()

    with tile.TileContext(nc) as tc, ExitStack() as ctx:
        sb1 = ctx.enter_context(tc.tile_pool(name="sb1", bufs=1))
        sbp = ctx.enter_context(tc.tile_pool(name="sbp", bufs=10))
        sbo = ctx.enter_context(tc.tile_pool(name="sbo", bufs=4))
        sbe = ctx.enter_context(tc.tile_pool(name="sbe", bufs=4))
        ps_e = ctx.enter_context(tc.tile_pool(name="pse", bufs=5, space="PSUM"))
        ps_o = ctx.enter_context(tc.tile_pool(name="pso", bufs=1, space="PSUM"))
        ps_c = ctx.enter_context(tc.tile_pool(name="psc", bufs=1, space="PSUM"))

        ps_f = None
        args = (nc, sb1, sbp, sbo, sbe, ps_f, ps_e, ps_o, ps_c,
                x_d, xT_d, w1T_d, b1_d, y_d, f32, f32r, AF)
        if loop_reps is None:
            _emit_body(*args)
        else:
            with tc.For_i(0, loop_reps, 1,
                          hint_engines=(mybir.EngineType.PE,
                                        mybir.EngineType.Activation,
                                        mybir.EngineType.DVE)):
                _emit_body(*args)

    nc.compile()
    return nc


def _get_nc(loop_reps=None):
    key = ("nc", loop_reps)
    if key not in _CACHE:
        _CACHE[key] = _build(loop_reps)
    return _CACHE[key]


def _make_in_maps(x, w1, b1):
    xf = np.ascontiguousarray(x.reshape(B, C, N), dtype=np.float32)
    w1T = np.ascontiguousarray(w1.T, dtype=np.float32)
    b1c = np.ascontiguousarray(b1.reshape(K, 1), dtype=np.float32)
    in_maps = []
    for core in range(N_CORES):
        b, h = divmod(core, 2)
        xs = xf[b] if h == 0 else np.roll(xf[b], -HALF, axis=1)
        in_maps.append({
            "x": np.ascontiguousarray(xs),
            "xT": np.ascontiguousarray(xs.T),
            "w1T": w1T,
            "b1": b1c,
        })
    return in_maps


def kernel(x, w1, b1):
    from concourse.bass_utils import run_bass_kernel_spmd

    nc = _get_nc()
    in_maps = _make_in_maps(x, w1, b1)
    res = run_bass_kernel_spmd(nc, in_maps, list(range(N_CORES)))
    out = np.empty((B, C, N), np.float32)
    for core in range(N_CORES):
        b, h = divmod(core, 2)
        out[b, :, h * HALF:(h + 1) * HALF] = res.results[core]["y"]
    return out.reshape(x.shape).astype(x.dtype, copy=False)



# revision 2
# speedup vs baseline: 1.4346x; 1.4346x over previous
"""Trainium2 Bass kernel for the self-attention module:

    f = conv1x1(x)            # [B, 16, N]   (w1 @ x + b1)
    E = f^T f                 # [B, N, N]    (symmetric)
    A = softmax(E, axis=-1)
    y = x + 0.1 * (x @ A^T)   # out[b,c,n] = sum_m x[b,c,m] A[b,n,m]

Sharding: 8 cores = 4 batches x 2 halves of the N=4096 rows. Each core
gets the full x[b] (column-rolled so its 2048-row half sits first) and
produces yT = y[:, :2048]^T for that layout (host transposes back).

Device algorithm per core (transposed-output dataflow):
  - f = w1p^T @ x + b1p                        [128, 4096] (rows 16..127
                                                zero via zero-padded w1)
  - per 512-wide n-block j, per PAIR of 128-wide m-chunks (i0,i1):
      E_psum[:, 0:512]    = f[:,i0]^T @ f[:,nsl]   (two banks, one tile)
      E_psum[:, 512:1024] = f[:,i1]^T @ f[:,nsl]
      p = exp(E_psum)     -> SBUF bf16             (ONE ScalarE instr per
                                                    pair: amortizes the
                                                    352-cyc ACT overhead)
      for each 128-wide n-chunk jj of the block, for i in (i0,i1):
        outT_psum[jj][n,c'] += p[:, i-part, jj*128:+128]^T @ xTb[:, i, :]
      where xTb = [x^T | 10.0] is [m, 257] bf16: column 256 of ones*10
      makes outT[:,256] = 10*colsum -- softmax denominator for FREE.
  - epilogue per jj: rec = 1/outT[:,256]  (per-partition scalar!)
      yT[n,c] = xT32[n,c] + outT[n,c]*rec          (= x + 0.1*out/colsum)

No colsum matmul (was 25% of PE work in the old layout), no gpsimd
partition-broadcast, exp at 2-bank granularity, p/xTb in bf16 so weight
loads use Fast Weight Load and stream fully overlapped.
"""

import numpy as np
import ml_dtypes

B, C, N = 4, 256, 64 * 64
K = 16
HALF = N // 2          # rows per core
NB = HALF // 512       # 4 n-blocks of 512
MC = N // 128          # 32 m-chunks of 128
N_CORES = 8

_CACHE: dict = {}


def _emit_body(nc, sb1, sbp, sbo, sbe, ps_e, ps_o,
               x_d, xTb_d, xT32_d, w1T_d, b1_d, y_d, f32, f32r, bf16, AF):
    # ---- load inputs ----
    # sync ring: w1, b1, xf (needed first), y-out later
    # scalar ring: xTb chunks then xT32 chunks
    w1T = sb1.tile([128, 2, 128], f32r, tag="w1T", bufs=2)
    nc.sync.dma_start(out=w1T,
                      in_=w1T_d.rearrange("(cc p) k -> p cc k", p=128).bitcast(f32r))
    b1 = sb1.tile([128, 1], f32, tag="b1", bufs=2)
    nc.sync.dma_start(out=b1, in_=b1_d)
    xf0 = sb1.tile([128, N], f32r, tag="xf0", bufs=2)
    xf1 = sb1.tile([128, N], f32r, tag="xf1", bufs=2)
    for q in range(4):
        qsl = slice(q * 1024, (q + 1) * 1024)
        nc.sync.dma_start(out=xf0[:, qsl], in_=x_d[0:128, qsl].bitcast(f32r))
        nc.sync.dma_start(out=xf1[:, qsl], in_=x_d[128:256, qsl].bitcast(f32r))
    xTb = sb1.tile([128, MC, 257], bf16, tag="xTb", bufs=2)
    for i in range(MC):
        nc.scalar.dma_start(out=xTb[:, i, :],
                            in_=xTb_d[i * 128:(i + 1) * 128, :])
    xT32 = sb1.tile([128, 16, C], f32, tag="xT32", bufs=2)
    for jj in range(16):
        nc.scalar.dma_start(out=xT32[:, jj, :],
                            in_=xT32_d[jj * 128:(jj + 1) * 128, :])

    # ---- f = w1 @ x + b1 : [128, N] (rows 16.. are zero) ----
    f_sb = sb1.tile([128, N], bf16, tag="f", bufs=2)
    for mj in range(N // 512):
        fp = ps_o.tile([128, 512], f32, tag="o")
        nc.tensor.matmul(fp, lhsT=w1T[:, 0, :],
                         rhs=xf0[:, mj * 512:(mj + 1) * 512],
                         start=True, stop=False)
        nc.tensor.matmul(fp, lhsT=w1T[:, 1, :],
                         rhs=xf1[:, mj * 512:(mj + 1) * 512],
                         start=False, stop=True)
        nc.vector.tensor_scalar_add(
            out=f_sb[:, mj * 512:(mj + 1) * 512], in0=fp, scalar1=b1)

    # ---- main: attention, transposed-output dataflow ----
    for j in range(NB):
        nsl = slice(j * 512, (j + 1) * 512)
        outs = []
        for jj in range(4):
            o = ps_o.tile([128, 257], f32, tag="o", name=f"out_{j}_{jj}")
            outs.append(o)
        for g in range(MC // 2):
            i0, i1 = 2 * g, 2 * g + 1
            ep = ps_e.tile([128, 1024], f32, tag="e")
            nc.tensor.matmul(ep[:, 0:512],
                             lhsT=f_sb[:, i0 * 128:(i0 + 1) * 128],
                             rhs=f_sb[:, nsl], start=True, stop=True)
            nc.tensor.matmul(ep[:, 512:1024],
                             lhsT=f_sb[:, i1 * 128:(i1 + 1) * 128],
                             rhs=f_sb[:, nsl], start=True, stop=True)
            p = sbp.tile([128, 1024], bf16, tag="p")
            nc.scalar.activation(out=p, in_=ep, func=AF.Exp)
            for k, i in ((0, i0), (1, i1)):
                for jj in range(4):
                    nc.tensor.matmul(
                        outs[jj],
                        lhsT=p[:, k * 512 + jj * 128:k * 512 + (jj + 1) * 128],
                        rhs=xTb[:, i, :],
                        start=(i == 0), stop=(i == MC - 1))
        # epilogue: yT[n, c] = xT32[n, c] + outT[n, c] / (10*colsum[n])
        for jj in range(4):
            nj = j * 4 + jj
            rec = sbe.tile([128, 1], f32, tag="rec")
            nc.vector.reciprocal(out=rec, in_=outs[jj][:, 256:257])
            yo = sbo.tile([128, C], f32, tag="yo")
            nc.vector.tensor_scalar_mul(out=yo, in0=outs[jj][:, 0:256],
                                        scalar1=rec)
            nc.vector.tensor_add(yo, yo, xT32[:, nj, :])
            nc.sync.dma_start(out=y_d[nj * 128:(nj + 1) * 128, :], in_=yo)


def _build(loop_reps=None):
    from contextlib import ExitStack

    import concourse.mybir as mybir
    import concourse.tile as tile
    from concourse import bacc

    f32 = mybir.dt.float32
    f32r = mybir.dt.float32r
    bf16 = mybir.dt.bfloat16
    AF = mybir.ActivationFunctionType

    nc = bacc.Bacc("TRN2", target_bir_lowering=False, debug=False,
                   num_devices=N_CORES)
    x_d = nc.dram_tensor("x", [C, N], f32, kind="ExternalInput").ap()
    xTb_d = nc.dram_tensor("xTb", [N, 257], bf16, kind="ExternalInput").ap()
    xT32_d = nc.dram_tensor("xT32", [HALF, C], f32, kind="ExternalInput").ap()
    w1T_d = nc.dram_tensor("w1T", [C, 128], f32, kind="ExternalInput").ap()
    b1_d = nc.dram_tensor("b1", [128, 1], f32, kind="ExternalInput").ap()
    y_d = nc.dram_tensor("y", [HALF, C], f32, kind="ExternalOutput").ap()

    with tile.TileContext(nc) as tc, ExitStack() as ctx:
        sb1 = ctx.enter_context(tc.tile_pool(name="sb1", bufs=1))
        sbp = ctx.enter_context(tc.tile_pool(name="sbp", bufs=3))
        sbo = ctx.enter_context(tc.tile_pool(name="sbo", bufs=4))
        sbe = ctx.enter_context(tc.tile_pool(name="sbe", bufs=4))
        ps_e = ctx.enter_context(tc.tile_pool(name="pse", bufs=2, space="PSUM"))
        ps_o = ctx.enter_context(tc.tile_pool(name="pso", bufs=4, space="PSUM"))

        args = (nc, sb1, sbp, sbo, sbe, ps_e, ps_o,
                x_d, xTb_d, xT32_d, w1T_d, b1_d, y_d, f32, f32r, bf16, AF)
        if loop_reps is None:
            _emit_body(*args)
        else:
            with tc.For_i(0, loop_reps, 1,
                          hint_engines=(mybir.EngineType.PE,
                                        mybir.EngineType.Activation,
                                        mybir.EngineType.DVE)):
                _emit_body(*args)

    nc.compile()
    return nc


def _get_nc(loop_reps=None):
    key = ("nc", loop_reps)
    if key not in _CACHE:
        _CACHE[key] = _build(loop_reps)
    return _CACHE[key]


def _make_in_maps(x, w1, b1):
    xf = np.ascontiguousarray(x.reshape(B, C, N), dtype=np.float32)
    w1Tp = np.zeros((C, 128), dtype=np.float32)
    w1Tp[:, :K] = np.asarray(w1, dtype=np.float32).T
    b1p = np.zeros((128, 1), dtype=np.float32)
    b1p[:K, 0] = np.asarray(b1, dtype=np.float32)
    in_maps = []
    for core in range(N_CORES):
        b, h = divmod(core, 2)
        xs = xf[b] if h == 0 else np.roll(xf[b], -HALF, axis=1)
        xsT = xs.T  # [N, C]
        xTb = np.empty((N, 257), dtype=ml_dtypes.bfloat16)
        xTb[:, :256] = xsT.astype(ml_dtypes.bfloat16)
        xTb[:, 256] = np.float32(10.0)
        in_maps.append({
            "x": np.ascontiguousarray(xs),
            "xTb": xTb,
            "xT32": np.ascontiguousarray(xsT[:HALF], dtype=np.float32),
            "w1T": w1Tp,
            "b1": b1p,
        })
    return in_maps


def kernel(x, w1, b1):
    from concourse.bass_utils import run_bass_kernel_spmd

    nc = _get_nc()
    in_maps = _make_in_maps(x, w1, b1)
    res = run_bass_kernel_spmd(nc, in_maps, list(range(N_CORES)))
    out = np.empty((B, C, N), np.float32)
    for core in range(N_CORES):
        b, h = divmod(core, 2)
        out[b, :, h * HALF:(h + 1) * HALF] = res.results[core]["y"].T
    return out.reshape(x.shape).astype(x.dtype, copy=False)
